# revision 15
# baseline (speedup 1.0000x reference)
"""Trainium2 Bass kernel for nn_ADI_16389595202112 (moe_routing, 8 cores).

Data-parallel over batch (2048 samples/core, no collectives).
Host: fold BN into weights, fold domain emb into ADI bias, slice domain-d
task weights, pack per-side blobs; convert matmul-facing data to bf16.
Device (feature-major, N=512 chunks): indirect-DMA embedding gathers,
PE transposes, bf16 matmuls (f32 PSUM), softmax via Exp + ones-matmul +
broadcast reciprocal, gate rows broadcast via gpsimd partition_broadcast,
sigmoid computed as 1/(1+exp(-x)) on broadcast tiles (avoids ACT table
switches), fused relu-mul evictions on DVE, biases via K=1 rank-1
matmuls into PSUM where fused.
"""

import sys

import numpy as np

if "/opt/trn_rl_repo" not in sys.path:
    sys.path.insert(0, "/opt/trn_rl_repo")

import ml_dtypes

import concourse.bass as bass
import concourse.mybir as mybir
import concourse.tile as tile
from concourse import bacc
from concourse.bass import IndirectOffsetOnAxis
from concourse.bass_utils import run_bass_kernel_spmd

EPS = 1e-5
B, L = 16384, 128
E, T = 8, 5
NUM_ROWS = 100000
NCORES = 8
BC = B // NCORES          # 2048 samples per core
NCH = 4                   # chunks per core
CN = BC // NCH            # 512 samples per chunk
NT = BC // 128            # 16 gather tiles per side

# weight blob column layout (bf16, [128, WCOLS])
W1_OFF = 0                # [128, 256]
SFC1_OFF = 256            # [128, 2048]
W2_OFF = 2304             # [128, 256]   (2 k-chunks of [128,128])
SFC2_OFF = 2560           # [128, 2048]  (blocks (e,kc) of [128,128])
W3_OFF = 4608             # [128, 576]   (3 k-chunks of [128,192])
W4_OFF = 5184             # [128, 64]    (k0 [128,32] | k1 rows<64 [64,32])
GATE_OFF = 5248           # [128, 8]
ADI_OFF = 5256            # [128, 2]
BROW_OFF = 5282           # partition-0 bias rows: bsfc2 (8*128) then b2 (128)
WCOLS = 5282 + 9 * 128

# bias blob column layout (f32, [128, BCOLS])
B1_C, BS1_C, B3_C, B4_C, GB_C, AB_C = 0, 2, 18, 20, 21, 22
BCOLS = 23

F32 = mybir.dt.float32
BF16 = mybir.dt.bfloat16
I32 = mybir.dt.int32

BF = ml_dtypes.bfloat16


def _fold_side(p, prefix, d, dom_emb):
    g = lambda n: np.asarray(p[prefix + n], dtype=np.float64)

    def bn_fold(W, b, bn):
        gamma, beta, mean, var = bn[0], bn[1], bn[2], bn[3]
        s = gamma / np.sqrt(var + EPS)
        return W * s[None, :], (b - mean) * s + beta

    W1, b1 = bn_fold(g("m1_w")[d], g("m1_b")[d], g("bn1")[d])
    W2, b2 = bn_fold(g("m2_w")[d], g("m2_b")[d], g("bn2")[d])
    Wsfc1, bsfc1, Wsfc2, bsfc2 = [], [], [], []
    for e in range(E):
        w, b = bn_fold(g("sfc1_w")[e], g("sfc1_b")[e], g("dsbn1")[d * E + e])
        Wsfc1.append(w)
        bsfc1.append(b)
        w, b = bn_fold(g("sfc2_w")[e], g("sfc2_b")[e], g("dsbn2")[d * E + e])
        Wsfc2.append(w)
        bsfc2.append(b)
    W3, b3 = bn_fold(g("m3_w")[d], g("m3_b")[d], g("bn3")[d])
    W4, b4 = g("m4_w")[d], g("m4_b")[d]
    adi_b = np.asarray(dom_emb, np.float64) @ g("adi_w") + g("adi_b")

    wblob = np.zeros((128, WCOLS), np.float64)
    wblob[:, W1_OFF:W1_OFF + 256] = W1
    wblob[:, SFC1_OFF:SFC1_OFF + 2048] = np.concatenate(Wsfc1, axis=1)
    for j in range(2):
        wblob[:, W2_OFF + j * 128:W2_OFF + (j + 1) * 128] = W2[j * 128:(j + 1) * 128]
    for e in range(E):
        for kc in range(2):
            c0 = SFC2_OFF + (2 * e + kc) * 128
            wblob[:, c0:c0 + 128] = Wsfc2[e][kc * 128:(kc + 1) * 128]
    for k in range(3):
        wblob[:, W3_OFF + k * 192:W3_OFF + (k + 1) * 192] = W3[k * 128:(k + 1) * 128]
    wblob[:, W4_OFF:W4_OFF + 32] = W4[0:128]
    wblob[0:64, W4_OFF + 32:W4_OFF + 64] = W4[128:192]
    wblob[:, GATE_OFF:GATE_OFF + 8] = g("gate_w")[d]
    wblob[:, ADI_OFF:ADI_OFF + 2] = g("adi_w")
    for e in range(E):
        wblob[0, BROW_OFF + e * 128:BROW_OFF + (e + 1) * 128] = bsfc2[e]
    wblob[0, BROW_OFF + 8 * 128:BROW_OFF + 9 * 128] = b2

    bblob = np.zeros((128, BCOLS), np.float64)
    bblob[:, B1_C] = b1[0:128]
    bblob[:, B1_C + 1] = b1[128:256]
    bsfc1 = np.concatenate(bsfc1)
    for m in range(16):
        bblob[:, BS1_C + m] = bsfc1[m * 128:(m + 1) * 128]
    bblob[:, B3_C] = b3[0:128]
    bblob[0:64, B3_C + 1] = b3[128:192]
    bblob[0:32, B4_C] = b4
    bblob[0:8, GB_C] = g("gate_b")[d]
    bblob[0:2, AB_C] = -adi_b          # Exp(scale=-1, bias=-adi_b)
    return wblob.astype(BF), bblob.astype(np.float32)


def _build_nc():
    nc = bacc.Bacc(None, target_bir_lowering=False)

    uidx_d = nc.declare_dram_parameter("uidx", [128, NT], I32, isOutput=False)
    iidx_d = nc.declare_dram_parameter("iidx", [128, NT], I32, isOutput=False)
    embu_d = nc.declare_dram_parameter("emb_user", [NUM_ROWS, L], BF16, isOutput=False)
    embi_d = nc.declare_dram_parameter("emb_item", [NUM_ROWS, L], BF16, isOutput=False)
    wu_d = nc.declare_dram_parameter("wu", [128, WCOLS], BF16, isOutput=False)
    wi_d = nc.declare_dram_parameter("wi", [128, WCOLS], BF16, isOutput=False)
    bu_d = nc.declare_dram_parameter("bu", [128, BCOLS], F32, isOutput=False)
    bi_d = nc.declare_dram_parameter("bi", [128, BCOLS], F32, isOutput=False)
    ones_d = nc.declare_dram_parameter("onesc", [128, CN], BF16, isOutput=False)
    sel2_d = nc.declare_dram_parameter("sel2", [3, 256], BF16, isOutput=False)
    idn_d = nc.declare_dram_parameter("idn", [128, 128], BF16, isOutput=False)
    out_d = nc.declare_dram_parameter("out", [1, BC], F32, isOutput=True)

    A = mybir.ActivationFunctionType
    OP = mybir.AluOpType

    with nc.allow_low_precision(reason="bf16 activations, f32 psum accum"), \
         tile.TileContext(nc) as tc:
        with (
            tc.tile_pool(name="const", bufs=1) as cp,
            tc.tile_pool(name="gath", bufs=16) as gp,
            tc.tile_pool(name="work", bufs=2) as wp,
            tc.tile_pool(name="s1p", bufs=18) as s1p,
            tc.tile_pool(name="pbig", bufs=6, space="PSUM") as pb,
            tc.tile_pool(name="psml", bufs=1, space="PSUM") as ps,
        ):
            ones = cp.tile([128, CN], BF16)
            nc.sync.dma_start(out=ones[:], in_=ones_d[:, :])
            ident = cp.tile([128, 128], BF16)
            nc.sync.dma_start(out=ident[:], in_=idn_d[:, :])
            sel2 = cp.tile([3, 256], BF16)
            nc.sync.dma_start(out=sel2[:], in_=sel2_d[:, :])

            sides = []
            for name, wd, bd, xd, ed in (
                ("u", wu_d, bu_d, uidx_d, embu_d),
                ("i", wi_d, bi_d, iidx_d, embi_d),
            ):
                idx = cp.tile([128, NT], I32, name=f"idx_{name}")
                nc.sync.dma_start(out=idx[:], in_=xd[:, :])
                w = cp.tile([128, WCOLS], BF16, name=f"w_{name}")
                b = cp.tile([128, BCOLS], F32, name=f"b_{name}")
                sout = cp.tile([32, BC], BF16, name=f"sout_{name}")
                sides.append((w, b, idx, ed, sout))
            for name, wd, bd, (w, b, idx, ed, sout) in (
                ("u", wu_d, bu_d, sides[0]),
                ("i", wi_d, bi_d, sides[1]),
            ):
                nc.sync.dma_start(out=b[:], in_=bd[:, :])
                nc.sync.dma_start(out=w[:], in_=wd[:, :])

            gathered = {}

            def emit_gathers_chunk(s, c):
                w, b, idx, ed, sout = sides[s]
                for t in range(4 * c, 4 * c + 4):
                    gt = gp.tile([128, 128], BF16, tag="g", name=f"g{s}_{t}")
                    nc.gpsimd.indirect_dma_start(
                        out=gt[:], out_offset=None, in_=ed[:, :],
                        in_offset=IndirectOffsetOnAxis(ap=idx[:, t:t + 1], axis=0))
                    gathered[(s, t)] = gt

            state = {}

            def emit_A(s, c):
                w, b, idx, ed, sout = sides[s]

                # embT: 4 PE transposes into one psum bank, 1 copy out
                embT = wp.tile([128, CN], BF16, tag="embT")
                tp = pb.tile([128, CN], BF16, tag="big", name="tp")
                for t in range(4):
                    nc.tensor.transpose(tp[:, t * 128:(t + 1) * 128],
                                        gathered[(s, 4 * c + t)][:], ident[:])
                nc.vector.tensor_copy(embT[:], tp[:])

                # gate logits -> expg (Exp with bias)
                gps = ps.tile([8, CN], F32, tag="sml", name="gps")
                nc.tensor.matmul(gps[:], w[:, GATE_OFF:GATE_OFF + 8], embT[:],
                                 start=True, stop=True)
                expg = wp.tile([8, CN], BF16, tag="expg")
                nc.scalar.activation(expg[:], gps[:], A.Exp,
                                     bias=b[0:8, GB_C:GB_C + 1])
                # ADI gate pre-activation: eneg = exp(-(logits + adi_b))
                aps = ps.tile([2, CN], F32, tag="sml", name="aps")
                nc.tensor.matmul(aps[:], w[:, ADI_OFF:ADI_OFF + 2], embT[:],
                                 start=True, stop=True)
                eneg = wp.tile([3, CN], BF16, tag="eneg", bufs=3)
                nc.scalar.activation(eneg[0:2, :], aps[:], A.Exp,
                                     bias=b[0:2, AB_C:AB_C + 1], scale=-1.0)
                nc.sync.dma_start(out=eneg[2:3, :], in_=ones[0:1, :])

                # gate rows -> partition 0 -> gpsimd broadcast
                bcg = []
                for e in range(E):
                    re_ = wp.tile([1, CN], BF16, tag="rowe", bufs=10, name=f"re{e}")
                    nc.sync.dma_start(out=re_[:], in_=expg[e:e + 1, :])
                    bce = wp.tile([128, CN], BF16, tag="bce", bufs=10, name=f"bce{e}")
                    nc.gpsimd.partition_broadcast(bce[:], re_[:])
                    bcg.append(bce)

                # m1 -> y1r (ACT relu+bias)
                y1r = wp.tile([128, 2 * CN], BF16, tag="y1r")
                for j in range(2):
                    mm = pb.tile([128, CN], F32, tag="big", name="mm_y1")
                    nc.tensor.matmul(mm[:],
                                     w[:, W1_OFF + j * 128:W1_OFF + (j + 1) * 128],
                                     embT[:], start=True, stop=True)
                    nc.scalar.activation(y1r[:, j * CN:(j + 1) * CN], mm[:], A.Relu,
                                         bias=b[:, B1_C + j:B1_C + j + 1])

                # sfc1 -> s1r (ACT relu+bias); emitted interleaved with sfc2
                s1r = []

                def emit_sfc1(m):
                    mm = pb.tile([128, CN], F32, tag="big", name="mm_s1")
                    nc.tensor.matmul(
                        mm[:], w[:, SFC1_OFF + m * 128:SFC1_OFF + (m + 1) * 128],
                        embT[:], start=True, stop=True)
                    t_ = s1p.tile([128, CN], BF16, tag="s1r", name=f"s1r{m}")
                    nc.scalar.activation(t_[:], mm[:], A.Relu,
                                         bias=b[:, BS1_C + m:BS1_C + m + 1])
                    s1r.append(t_)

                for m in range(8):
                    emit_sfc1(m)

                # gate-sum broadcast -> 1/x ; sigmoid broadcasts via PE + 1/x
                bgs = pb.tile([128, CN], F32, tag="big", name="bgs")
                nc.tensor.matmul(bgs[:], ones[0:8, 0:128], expg[:],
                                 start=True, stop=True)
                ginvb = wp.tile([128, CN], F32, tag="ginvb")
                nc.vector.reciprocal_approx_fast(ginvb[:], bgs[:])
                bgab = []
                for r in range(2):
                    zp = pb.tile([128, CN], F32, tag="big", name=f"zp{r}")
                    nc.tensor.matmul(zp[:], sel2[0:3, r * 128:(r + 1) * 128],
                                     eneg[:], start=True, stop=True)
                    bga = wp.tile([128, CN], F32, tag=f"bga{r}")
                    nc.vector.reciprocal_approx_fast(bga[:], zp[:])
                    bgab.append(bga)

                # sfc2 per expert (+bias row), fused relu*gate on DVE,
                # interleaved with remaining sfc1 blocks to hide evict latency
                prods = []
                for e in range(E):
                    mm = pb.tile([128, CN], F32, tag="big", name="mm_s2")
                    for kc in range(2):
                        c0 = SFC2_OFF + (2 * e + kc) * 128
                        nc.tensor.matmul(mm[:], w[:, c0:c0 + 128],
                                         s1r[2 * e + kc][:],
                                         start=(kc == 0), stop=False)
                    nc.tensor.matmul(mm[:],
                                     w[0:1, BROW_OFF + e * 128:BROW_OFF + (e + 1) * 128],
                                     ones[0:1, :], start=False, stop=True)
                    pe = wp.tile([128, CN], BF16, tag="gt", bufs=9, name=f"gt{e}")
                    nc.vector.scalar_tensor_tensor(pe[:], mm[:], 0.0, bcg[e][:],
                                                   op0=OP.max, op1=OP.mult)
                    prods.append(pe)
                    for m in (8 + 2 * e, 9 + 2 * e):
                        if m < 16:
                            emit_sfc1(m)
                while len(prods) > 1:
                    nxt = []
                    for k in range(0, len(prods), 2):
                        dst = prods[k]
                        nc.vector.tensor_add(dst[:], dst[:], prods[k + 1][:])
                        nxt.append(dst)
                    prods = nxt

                # m2 (+bias row) into psum, evicted in B
                mm2 = pb.tile([128, CN], F32, tag="m2ps", bufs=1, name="mm_m2")
                for j in range(2):
                    nc.tensor.matmul(mm2[:],
                                     w[:, W2_OFF + j * 128:W2_OFF + (j + 1) * 128],
                                     y1r[:, j * CN:(j + 1) * CN],
                                     start=(j == 0), stop=False)
                nc.tensor.matmul(mm2[:],
                                 w[0:1, BROW_OFF + 8 * 128:BROW_OFF + 9 * 128],
                                 ones[0:1, :], start=False, stop=True)

                # ADI combine (DVE): spec / share / spec*share
                acc = prods[0]
                spec = wp.tile([128, CN], BF16, tag="spec", bufs=3)
                nc.vector.scalar_tensor_tensor(spec[:], mm2[:], 0.0, bgab[1][:],
                                               op0=OP.max, op1=OP.mult)
                q = wp.tile([128, CN], BF16, tag="q", bufs=3)
                nc.vector.tensor_mul(q[:], ginvb[:], bgab[0][:])
                share = wp.tile([128, CN], BF16, tag="share", bufs=3)
                nc.vector.tensor_mul(share[:], acc[:], q[:])
                sshare = wp.tile([128, CN], BF16, tag="sshare", bufs=3)
                nc.vector.tensor_mul(sshare[:], spec[:], share[:])
                state[(s, c)] = (spec, share, sshare)

            def emit_B(s, c):
                w, b, idx, ed, sout = sides[s]
                spec, share, sshare = state.pop((s, c))

                # m3 (2 M-chunks x 3 k-chunks)
                h3r = wp.tile([128, 2 * CN], BF16, tag="h3r")
                for j, mw in ((0, 128), (1, 64)):
                    mm = pb.tile([128, CN], F32, tag="big", name="mm_m3")
                    for k, src in enumerate((spec, sshare, share)):
                        c0 = W3_OFF + k * 192 + j * 128
                        nc.tensor.matmul(mm[0:mw, :], w[:, c0:c0 + mw], src[:],
                                         start=(k == 0), stop=(k == 2))
                    nc.scalar.activation(h3r[0:mw, j * CN:(j + 1) * CN],
                                         mm[0:mw, :], A.Relu,
                                         bias=b[0:mw, B3_C + j:B3_C + j + 1])

                # m4 -> sout chunk
                mm = pb.tile([128, CN], F32, tag="big", name="mm_m4")
                nc.tensor.matmul(mm[0:32, :], w[0:128, W4_OFF:W4_OFF + 32],
                                 h3r[:, 0:CN], start=True, stop=False)
                nc.tensor.matmul(mm[0:32, :], w[0:64, W4_OFF + 32:W4_OFF + 64],
                                 h3r[0:64, CN:2 * CN], start=False, stop=True)
                nc.scalar.activation(sout[:, c * CN:(c + 1) * CN], mm[0:32, :],
                                     A.Relu, bias=b[0:32, B4_C:B4_C + 1])

            order = [(s, c) for s in range(2) for c in range(NCH)]
            emit_gathers_chunk(*order[0])
            emit_gathers_chunk(*order[1])
            for k, (s, c) in enumerate(order):
                if k + 2 < len(order):
                    emit_gathers_chunk(*order[k + 2])
                if k >= 1:
                    emit_B(*order[k - 1])
                emit_A(s, c)
            emit_B(*order[-1])

            # logits: sum over 32 features of u*i
            outsb = cp.tile([1, BC], F32)
            su, si = sides[0][4], sides[1][4]
            for c in range(NCH):
                cs = slice(c * CN, (c + 1) * CN)
                prod = wp.tile([32, CN], BF16, tag="prod")
                nc.vector.tensor_mul(prod[:], su[:, cs], si[:, cs])
                rp = ps.tile([1, CN], F32, tag="sml", name="rp")
                nc.tensor.matmul(rp[:], ones[0:32, 0:1], prod[:],
                                 start=True, stop=True)
                nc.scalar.activation(outsb[0:1, cs], rp[:], A.Copy)
            nc.sync.dma_start(out=out_d[:, :], in_=outsb[:])

    nc.finalize()
    return nc


def _make_in_maps(inputs):
    d = int(inputs["domain_idc"])
    wu, bu = _fold_side(inputs, "u_", d, np.asarray(inputs["domain_embs"])[d])
    wi, bi = _fold_side(inputs, "i_", d, np.asarray(inputs["domain_embs"])[d + T])
    v = lambda x: x.view(np.uint16)
    embu = v(np.ascontiguousarray(np.asarray(inputs["emb_user"], np.float32).astype(BF)))
    embi = v(np.ascontiguousarray(np.asarray(inputs["emb_item"], np.float32).astype(BF)))
    uidx = np.asarray(inputs["user_indices"], np.int32)
    iidx = np.asarray(inputs["item_indices"], np.int32)
    onesc = v(np.ones((128, CN), BF))
    idn = v(np.eye(128).astype(BF))
    sel2 = np.zeros((3, 256), BF)
    sel2[0, 0:128] = 1
    sel2[1, 128:256] = 1
    sel2[2, :] = 1
    sel2 = v(sel2)

    in_maps = []
    for c in range(NCORES):
        sl = slice(c * BC, (c + 1) * BC)
        in_maps.append({
            "uidx": np.ascontiguousarray(uidx[sl].reshape(NT, 128).T),
            "iidx": np.ascontiguousarray(iidx[sl].reshape(NT, 128).T),
            "emb_user": embu,
            "emb_item": embi,
            "wu": v(wu), "wi": v(wi), "bu": bu, "bi": bi,
            "onesc": onesc, "idn": idn, "sel2": sel2,
        })
    return in_maps


_CACHED_NC = None


def _get_nc():
    global _CACHED_NC
    if _CACHED_NC is None:
        _CACHED_NC = _build_nc()
    return _CACHED_NC


def run(inputs, **kw):
    """Run on 8 cores; returns (full_output, BassKernelResults)."""
    res = run_bass_kernel_spmd(_get_nc(), _make_in_maps(inputs),
                               core_ids=list(range(NCORES)), **kw)
    out = np.concatenate([res.results[c]["out"].reshape(-1)
                          for c in range(NCORES)])
    return out.reshape(B, 1).astype(np.float32), res


def kernel(**inputs):
    out, _ = run(inputs)
    return out


# revision 16
# speedup vs baseline: 1.2223x; 1.2223x over previous
"""Trainium2 Bass kernel for nn_ADI_16389595202112 (moe_routing, 8 cores).

Data-parallel over batch (2048 samples/core, no collectives).
Host: fold BN into weights, fold domain emb into ADI bias, slice domain-d
task weights, pack per-side blobs; convert matmul-facing data to bf16.
Device (feature-major, N=512 chunks): indirect-DMA embedding gathers,
PE transposes, bf16 matmuls (f32 PSUM), softmax via Exp + ones-matmul +
broadcast reciprocal, gate rows broadcast via gpsimd partition_broadcast,
sigmoid computed as 1/(1+exp(-x)) on broadcast tiles (avoids ACT table
switches), fused relu-mul evictions on DVE, biases via K=1 rank-1
matmuls into PSUM where fused.
"""

import sys

import numpy as np

if "/opt/trn_rl_repo" not in sys.path:
    sys.path.insert(0, "/opt/trn_rl_repo")

import ml_dtypes

import concourse.bass as bass
import concourse.mybir as mybir
import concourse.tile as tile
from concourse import bacc
from concourse.bass import IndirectOffsetOnAxis
from concourse.bass_utils import run_bass_kernel_spmd

EPS = 1e-5
B, L = 16384, 128
E, T = 8, 5
NUM_ROWS = 100000
NCORES = 8
BC = B // NCORES          # 2048 samples per core
NCH = 4                   # chunks per core
CN = BC // NCH            # 512 samples per chunk
NT = BC // 128            # 16 gather tiles per side

# weight blob column layout (bf16, [128, WCOLS])
W1_OFF = 0                # [128, 256]
SFC1_OFF = 256            # [128, 2048]
W2_OFF = 2304             # [128, 256]   (2 k-chunks of [128,128])
SFC2_OFF = 2560           # [128, 2048]  (blocks (e,kc) of [128,128])
W3_OFF = 4608             # [128, 576]   (3 k-chunks of [128,192])
W4_OFF = 5184             # [128, 64]    (k0 [128,32] | k1 rows<64 [64,32])
GATE_OFF = 5248           # [128, 8]
ADI_OFF = 5256            # [128, 2]
BROW_OFF = 5282           # partition-0 bias rows: bsfc2 (8*128) then b2 (128)
WCOLS = 5282 + 9 * 128

# bias blob column layout (f32, [128, BCOLS])
B1_C, BS1_C, B3_C, B4_C, GB_C, AB_C = 0, 2, 18, 20, 21, 22
BCOLS = 23

F32 = mybir.dt.float32
BF16 = mybir.dt.bfloat16
I32 = mybir.dt.int32

BF = ml_dtypes.bfloat16


def _fold_side(p, prefix, d, dom_emb):
    g = lambda n: np.asarray(p[prefix + n], dtype=np.float64)

    def bn_fold(W, b, bn):
        gamma, beta, mean, var = bn[0], bn[1], bn[2], bn[3]
        s = gamma / np.sqrt(var + EPS)
        return W * s[None, :], (b - mean) * s + beta

    W1, b1 = bn_fold(g("m1_w")[d], g("m1_b")[d], g("bn1")[d])
    W2, b2 = bn_fold(g("m2_w")[d], g("m2_b")[d], g("bn2")[d])
    Wsfc1, bsfc1, Wsfc2, bsfc2 = [], [], [], []
    for e in range(E):
        w, b = bn_fold(g("sfc1_w")[e], g("sfc1_b")[e], g("dsbn1")[d * E + e])
        Wsfc1.append(w)
        bsfc1.append(b)
        w, b = bn_fold(g("sfc2_w")[e], g("sfc2_b")[e], g("dsbn2")[d * E + e])
        Wsfc2.append(w)
        bsfc2.append(b)
    W3, b3 = bn_fold(g("m3_w")[d], g("m3_b")[d], g("bn3")[d])
    W4, b4 = g("m4_w")[d], g("m4_b")[d]
    adi_b = np.asarray(dom_emb, np.float64) @ g("adi_w") + g("adi_b")

    wblob = np.zeros((128, WCOLS), np.float64)
    wblob[:, W1_OFF:W1_OFF + 256] = W1
    wblob[:, SFC1_OFF:SFC1_OFF + 2048] = np.concatenate(Wsfc1, axis=1)
    for j in range(2):
        wblob[:, W2_OFF + j * 128:W2_OFF + (j + 1) * 128] = W2[j * 128:(j + 1) * 128]
    for e in range(E):
        for kc in range(2):
            c0 = SFC2_OFF + (2 * e + kc) * 128
            wblob[:, c0:c0 + 128] = Wsfc2[e][kc * 128:(kc + 1) * 128]
    for k in range(3):
        wblob[:, W3_OFF + k * 192:W3_OFF + (k + 1) * 192] = W3[k * 128:(k + 1) * 128]
    wblob[:, W4_OFF:W4_OFF + 32] = W4[0:128]
    wblob[0:64, W4_OFF + 32:W4_OFF + 64] = W4[128:192]
    wblob[:, GATE_OFF:GATE_OFF + 8] = g("gate_w")[d]
    wblob[:, ADI_OFF:ADI_OFF + 2] = g("adi_w")
    for e in range(E):
        wblob[0, BROW_OFF + e * 128:BROW_OFF + (e + 1) * 128] = bsfc2[e]
    wblob[0, BROW_OFF + 8 * 128:BROW_OFF + 9 * 128] = b2

    bblob = np.zeros((128, BCOLS), np.float64)
    bblob[:, B1_C] = b1[0:128]
    bblob[:, B1_C + 1] = b1[128:256]
    bsfc1 = np.concatenate(bsfc1)
    for m in range(16):
        bblob[:, BS1_C + m] = bsfc1[m * 128:(m + 1) * 128]
    bblob[:, B3_C] = b3[0:128]
    bblob[0:64, B3_C + 1] = b3[128:192]
    bblob[0:32, B4_C] = b4
    bblob[0:8, GB_C] = g("gate_b")[d]
    bblob[0:2, AB_C] = -adi_b          # Exp(scale=-1, bias=-adi_b)
    return wblob.astype(BF), bblob.astype(np.float32)


def _build_nc():
    nc = bacc.Bacc(None, target_bir_lowering=False)

    uidx_d = nc.declare_dram_parameter("uidx", [128, NT], I32, isOutput=False)
    iidx_d = nc.declare_dram_parameter("iidx", [128, NT], I32, isOutput=False)
    embu_d = nc.declare_dram_parameter("emb_user", [NUM_ROWS, L], BF16, isOutput=False)
    embi_d = nc.declare_dram_parameter("emb_item", [NUM_ROWS, L], BF16, isOutput=False)
    wu_d = nc.declare_dram_parameter("wu", [128, WCOLS], BF16, isOutput=False)
    wi_d = nc.declare_dram_parameter("wi", [128, WCOLS], BF16, isOutput=False)
    bu_d = nc.declare_dram_parameter("bu", [128, BCOLS], F32, isOutput=False)
    bi_d = nc.declare_dram_parameter("bi", [128, BCOLS], F32, isOutput=False)
    ones_d = nc.declare_dram_parameter("onesc", [128, CN], BF16, isOutput=False)
    sel2_d = nc.declare_dram_parameter("sel2", [3, 256], BF16, isOutput=False)
    idn_d = nc.declare_dram_parameter("idn", [128, 128], BF16, isOutput=False)
    out_d = nc.declare_dram_parameter("out", [1, BC], F32, isOutput=True)

    A = mybir.ActivationFunctionType
    OP = mybir.AluOpType

    with nc.allow_low_precision(reason="bf16 activations, f32 psum accum"), \
         tile.TileContext(nc) as tc:
        with (
            tc.tile_pool(name="const", bufs=1) as cp,
            tc.tile_pool(name="gath", bufs=16) as gp,
            tc.tile_pool(name="work", bufs=2) as wp,
            tc.tile_pool(name="s1p", bufs=18) as s1p,
            tc.tile_pool(name="pbig", bufs=6, space="PSUM") as pb,
            tc.tile_pool(name="psml", bufs=1, space="PSUM") as ps,
        ):
            ones = cp.tile([128, CN], BF16)
            nc.sync.dma_start(out=ones[:], in_=ones_d[:, :])
            ident = cp.tile([128, 128], BF16)
            nc.sync.dma_start(out=ident[:], in_=idn_d[:, :])
            sel2 = cp.tile([3, 256], BF16)
            nc.sync.dma_start(out=sel2[:], in_=sel2_d[:, :])

            sides = []
            for name, wd, bd, xd, ed in (
                ("u", wu_d, bu_d, uidx_d, embu_d),
                ("i", wi_d, bi_d, iidx_d, embi_d),
            ):
                idx = cp.tile([128, NT], I32, name=f"idx_{name}")
                nc.sync.dma_start(out=idx[:], in_=xd[:, :])
                w = cp.tile([128, WCOLS], BF16, name=f"w_{name}")
                b = cp.tile([128, BCOLS], F32, name=f"b_{name}")
                sout = cp.tile([32, BC], BF16, name=f"sout_{name}")
                sides.append((w, b, idx, ed, sout))
            for name, wd, bd, (w, b, idx, ed, sout) in (
                ("u", wu_d, bu_d, sides[0]),
                ("i", wi_d, bi_d, sides[1]),
            ):
                nc.sync.dma_start(out=b[:], in_=bd[:, :])
                nc.sync.dma_start(out=w[:], in_=wd[:, :])

            gathered = {}

            def emit_gathers_chunk(s, c):
                w, b, idx, ed, sout = sides[s]
                for t in range(4 * c, 4 * c + 4):
                    gt = gp.tile([128, 128], BF16, tag="g", name=f"g{s}_{t}")
                    nc.gpsimd.indirect_dma_start(
                        out=gt[:], out_offset=None, in_=ed[:, :],
                        in_offset=IndirectOffsetOnAxis(ap=idx[:, t:t + 1], axis=0))
                    gathered[(s, t)] = gt

            state = {}

            def emit_A(s, c):
                w, b, idx, ed, sout = sides[s]

                # embT: 4 PE transposes into one psum bank, 1 copy out
                embT = wp.tile([128, CN], BF16, tag="embT")
                tp = pb.tile([128, CN], BF16, tag="big", name="tp")
                for t in range(4):
                    nc.tensor.transpose(tp[:, t * 128:(t + 1) * 128],
                                        gathered[(s, 4 * c + t)][:], ident[:])
                nc.vector.tensor_copy(embT[:], tp[:])

                # gate logits -> expg (Exp with bias)
                gps = ps.tile([8, CN], F32, tag="sml", name="gps")
                nc.tensor.matmul(gps[:], w[:, GATE_OFF:GATE_OFF + 8], embT[:],
                                 start=True, stop=True)
                expg = wp.tile([8, CN], BF16, tag="expg")
                nc.scalar.activation(expg[:], gps[:], A.Exp,
                                     bias=b[0:8, GB_C:GB_C + 1])
                # ADI gate pre-activation: eneg = exp(-(logits + adi_b))
                aps = ps.tile([2, CN], F32, tag="sml", name="aps")
                nc.tensor.matmul(aps[:], w[:, ADI_OFF:ADI_OFF + 2], embT[:],
                                 start=True, stop=True)
                eneg = wp.tile([3, CN], BF16, tag="eneg", bufs=3)
                nc.scalar.activation(eneg[0:2, :], aps[:], A.Exp,
                                     bias=b[0:2, AB_C:AB_C + 1], scale=-1.0)
                nc.sync.dma_start(out=eneg[2:3, :], in_=ones[0:1, :])

                # gate rows -> partition 0 -> gpsimd broadcast
                bcg = []
                for e in range(E):
                    re_ = wp.tile([1, CN], BF16, tag="rowe", bufs=10, name=f"re{e}")
                    nc.sync.dma_start(out=re_[:], in_=expg[e:e + 1, :])
                    bce = wp.tile([128, CN], BF16, tag="bce", bufs=10, name=f"bce{e}")
                    nc.gpsimd.partition_broadcast(bce[:], re_[:])
                    bcg.append(bce)

                # m1 -> y1r (ACT relu+bias)
                y1r = wp.tile([128, 2 * CN], BF16, tag="y1r")
                for j in range(2):
                    mm = pb.tile([128, CN], F32, tag="big", name="mm_y1")
                    nc.tensor.matmul(mm[:],
                                     w[:, W1_OFF + j * 128:W1_OFF + (j + 1) * 128],
                                     embT[:], start=True, stop=True)
                    nc.scalar.activation(y1r[:, j * CN:(j + 1) * CN], mm[:], A.Relu,
                                         bias=b[:, B1_C + j:B1_C + j + 1])

                # sfc1 -> s1r (ACT relu+bias); emitted interleaved with sfc2
                s1r = []

                def emit_sfc1(m):
                    mm = pb.tile([128, CN], F32, tag="big", name="mm_s1")
                    nc.tensor.matmul(
                        mm[:], w[:, SFC1_OFF + m * 128:SFC1_OFF + (m + 1) * 128],
                        embT[:], start=True, stop=True)
                    t_ = s1p.tile([128, CN], BF16, tag="s1r", name=f"s1r{m}")
                    nc.scalar.activation(t_[:], mm[:], A.Relu,
                                         bias=b[:, BS1_C + m:BS1_C + m + 1])
                    s1r.append(t_)

                for m in range(8):
                    emit_sfc1(m)

                # gate-sum broadcast -> 1/x ; sigmoid broadcasts via PE + 1/x
                bgs = pb.tile([128, CN], F32, tag="big", name="bgs")
                nc.tensor.matmul(bgs[:], ones[0:8, 0:128], expg[:],
                                 start=True, stop=True)
                ginvb = wp.tile([128, CN], F32, tag="ginvb")
                nc.vector.reciprocal_approx_fast(ginvb[:], bgs[:])
                bgab = []
                for r in range(2):
                    zp = pb.tile([128, CN], F32, tag="big", name=f"zp{r}")
                    nc.tensor.matmul(zp[:], sel2[0:3, r * 128:(r + 1) * 128],
                                     eneg[:], start=True, stop=True)
                    bga = wp.tile([128, CN], F32, tag=f"bga{r}")
                    nc.vector.reciprocal_approx_fast(bga[:], zp[:])
                    bgab.append(bga)

                # sfc2 per expert (+bias row), fused relu*gate on DVE,
                # interleaved with remaining sfc1 blocks to hide evict latency
                prods = []
                for e in range(E):
                    mm = pb.tile([128, CN], F32, tag="big", name="mm_s2")
                    for kc in range(2):
                        c0 = SFC2_OFF + (2 * e + kc) * 128
                        nc.tensor.matmul(mm[:], w[:, c0:c0 + 128],
                                         s1r[2 * e + kc][:],
                                         start=(kc == 0), stop=False)
                    nc.tensor.matmul(mm[:],
                                     w[0:1, BROW_OFF + e * 128:BROW_OFF + (e + 1) * 128],
                                     ones[0:1, :], start=False, stop=True)
                    pe = wp.tile([128, CN], BF16, tag="gt", bufs=9, name=f"gt{e}")
                    nc.vector.scalar_tensor_tensor(pe[:], mm[:], 0.0, bcg[e][:],
                                                   op0=OP.max, op1=OP.mult)
                    prods.append(pe)
                    for m in (8 + 2 * e, 9 + 2 * e):
                        if m < 16:
                            emit_sfc1(m)
                while len(prods) > 1:
                    nxt = []
                    for k in range(0, len(prods), 2):
                        dst = prods[k]
                        nc.vector.tensor_add(dst[:], dst[:], prods[k + 1][:])
                        nxt.append(dst)
                    prods = nxt

                # m2 (+bias row) into psum, evicted in B
                mm2 = pb.tile([128, CN], F32, tag="m2ps", bufs=1, name="mm_m2")
                for j in range(2):
                    nc.tensor.matmul(mm2[:],
                                     w[:, W2_OFF + j * 128:W2_OFF + (j + 1) * 128],
                                     y1r[:, j * CN:(j + 1) * CN],
                                     start=(j == 0), stop=False)
                nc.tensor.matmul(mm2[:],
                                 w[0:1, BROW_OFF + 8 * 128:BROW_OFF + 9 * 128],
                                 ones[0:1, :], start=False, stop=True)

                # ADI combine (DVE): spec / share / spec*share
                acc = prods[0]
                spec = wp.tile([128, CN], BF16, tag="spec", bufs=3)
                nc.vector.scalar_tensor_tensor(spec[:], mm2[:], 0.0, bgab[1][:],
                                               op0=OP.max, op1=OP.mult)
                q = wp.tile([128, CN], BF16, tag="q", bufs=3)
                nc.vector.tensor_mul(q[:], ginvb[:], bgab[0][:])
                share = wp.tile([128, CN], BF16, tag="share", bufs=3)
                nc.vector.tensor_mul(share[:], acc[:], q[:])
                sshare = wp.tile([128, CN], BF16, tag="sshare", bufs=3)
                nc.vector.tensor_mul(sshare[:], spec[:], share[:])
                state[(s, c)] = (spec, share, sshare)

            def emit_B(s, c):
                w, b, idx, ed, sout = sides[s]
                spec, share, sshare = state.pop((s, c))

                # m3 (2 M-chunks x 3 k-chunks)
                h3r = wp.tile([128, 2 * CN], BF16, tag="h3r")
                for j, mw in ((0, 128), (1, 64)):
                    mm = pb.tile([128, CN], F32, tag="big", name="mm_m3")
                    for k, src in enumerate((spec, sshare, share)):
                        c0 = W3_OFF + k * 192 + j * 128
                        nc.tensor.matmul(mm[0:mw, :], w[:, c0:c0 + mw], src[:],
                                         start=(k == 0), stop=(k == 2))
                    nc.scalar.activation(h3r[0:mw, j * CN:(j + 1) * CN],
                                         mm[0:mw, :], A.Relu,
                                         bias=b[0:mw, B3_C + j:B3_C + j + 1])

                # m4 -> sout chunk
                mm = pb.tile([128, CN], F32, tag="big", name="mm_m4")
                nc.tensor.matmul(mm[0:32, :], w[0:128, W4_OFF:W4_OFF + 32],
                                 h3r[:, 0:CN], start=True, stop=False)
                nc.tensor.matmul(mm[0:32, :], w[0:64, W4_OFF + 32:W4_OFF + 64],
                                 h3r[0:64, CN:2 * CN], start=False, stop=True)
                nc.scalar.activation(sout[:, c * CN:(c + 1) * CN], mm[0:32, :],
                                     A.Relu, bias=b[0:32, B4_C:B4_C + 1])

            order = [(s, c) for s in range(2) for c in range(NCH)]
            emit_gathers_chunk(*order[0])
            emit_gathers_chunk(*order[1])
            for k, (s, c) in enumerate(order):
                if k + 2 < len(order):
                    emit_gathers_chunk(*order[k + 2])
                emit_A(s, c)
                if k >= 1:
                    emit_B(*order[k - 1])
            emit_B(*order[-1])

            # logits: sum over 32 features of u*i
            outsb = cp.tile([1, BC], F32)
            su, si = sides[0][4], sides[1][4]
            for c in range(NCH):
                cs = slice(c * CN, (c + 1) * CN)
                prod = wp.tile([32, CN], BF16, tag="prod")
                nc.vector.tensor_mul(prod[:], su[:, cs], si[:, cs])
                rp = ps.tile([1, CN], F32, tag="sml", name="rp")
                nc.tensor.matmul(rp[:], ones[0:32, 0:1], prod[:],
                                 start=True, stop=True)
                nc.scalar.activation(outsb[0:1, cs], rp[:], A.Copy)
            nc.sync.dma_start(out=out_d[:, :], in_=outsb[:])

    nc.finalize()
    return nc


def _make_in_maps(inputs):
    d = int(inputs["domain_idc"])
    wu, bu = _fold_side(inputs, "u_", d, np.asarray(inputs["domain_embs"])[d])
    wi, bi = _fold_side(inputs, "i_", d, np.asarray(inputs["domain_embs"])[d + T])
    v = lambda x: x.view(np.uint16)
    embu = v(np.ascontiguousarray(np.asarray(inputs["emb_user"], np.float32).astype(BF)))
    embi = v(np.ascontiguousarray(np.asarray(inputs["emb_item"], np.float32).astype(BF)))
    uidx = np.asarray(inputs["user_indices"], np.int32)
    iidx = np.asarray(inputs["item_indices"], np.int32)
    onesc = v(np.ones((128, CN), BF))
    idn = v(np.eye(128).astype(BF))
    sel2 = np.zeros((3, 256), BF)
    sel2[0, 0:128] = 1
    sel2[1, 128:256] = 1
    sel2[2, :] = 1
    sel2 = v(sel2)

    in_maps = []
    for c in range(NCORES):
        sl = slice(c * BC, (c + 1) * BC)
        in_maps.append({
            "uidx": np.ascontiguousarray(uidx[sl].reshape(NT, 128).T),
            "iidx": np.ascontiguousarray(iidx[sl].reshape(NT, 128).T),
            "emb_user": embu,
            "emb_item": embi,
            "wu": v(wu), "wi": v(wi), "bu": bu, "bi": bi,
            "onesc": onesc, "idn": idn, "sel2": sel2,
        })
    return in_maps


_CACHED_NC = None


def _get_nc():
    global _CACHED_NC
    if _CACHED_NC is None:
        _CACHED_NC = _build_nc()
    return _CACHED_NC


def run(inputs, **kw):
    """Run on 8 cores; returns (full_output, BassKernelResults)."""
    res = run_bass_kernel_spmd(_get_nc(), _make_in_maps(inputs),
                               core_ids=list(range(NCORES)), **kw)
    out = np.concatenate([res.results[c]["out"].reshape(-1)
                          for c in range(NCORES)])
    return out.reshape(B, 1).astype(np.float32), res


def kernel(**inputs):
    out, _ = run(inputs)
    return out
